# revision 3
# baseline (speedup 1.0000x reference)
"""Bivariate Gaussian kernel (Nadaraya-Watson) on 8 TRN2 NeuronCores — v3.

Baseline-proven PE structure (4-strip row-packed E matmuls, e4/e3 PSUM
rotation, shared-bank accumulators) with the exp stage split across the
scalar AND vector engines:

  z[n,m] = K16*E[n,m] + B16 in f32 PSUM (rank-11 compensated bf16 matmul,
  K16 = 128/ln2, B16 = int16 bits of bf16(128.0)), so z IS the
  bf16-Schraudolph code of W = 128*exp(E).
  - scalar groups: W = Exp(z*1/K16 - B16/K16 + ln128) -> bf16 (exact)
  - vector groups: W-bits = int16(max(z + DELTA, 0)) (PWL sawtooth ~±2%)
  n is pre-sorted by outputs[n]; vector groups take the smallest-|out|
  tiles, where the sawtooth barely affects t/s (t-weights near zero).
  Reduce: acc[0:3,m] += [1, out_hi, out_lo]^T @ W per n-tile (bf16, f32
  PSUM accumulation over all 64 tiles); host computes t/s.
"""

import functools
import math
import sys

import numpy as np

sys.path.insert(0, "/opt/trn_rl_repo")

EPS = 1e-7
N = 8192
M = 8192
NCORES = 8
MLOC = M // NCORES  # 1024
P = 128
NT = N // P  # 64 n-tiles
MBW = 512  # m-block width (one PSUM bank)
MB = MLOC // MBW  # 2 m-blocks
K = 11  # compensated-split rank

K16 = 128.0 / math.log(2.0)  # 184.6644
B16 = 17152.0  # int16 bits of bf16(128.0)
DELTA_SAW = -7.0  # sawtooth centering shift (z units)
SCALE_EXP = 1.0 / K16
BIAS_EXP = -B16 / K16 + math.log(128.0)

# per-half n-tile grouping: alternates the 4-bank and 3-bank PSUM buffers
HALF_SIZES = [1, 2] + [4, 3] * 8 + [4, 1]
assert sum(HALF_SIZES) == NT and len(HALF_SIZES) % 2 == 0


def _half_groups():
    out = []
    pos = 0
    for sz in HALF_SIZES:
        out.append(list(range(pos, pos + sz)))
        pos += sz
    return out


@functools.lru_cache(maxsize=1)
def _eng_order():
    """Strict S/D alternation: no same-engine cast runs, so neither engine
    ever backs up the PSUM rotation (the greedy order stalled the PE tail)."""
    return ["S", "D"] * (len(HALF_SIZES) // 2)


@functools.lru_cache(maxsize=1)
def _build():
    import concourse.tile as tile
    from concourse import bacc, mybir

    f32 = mybir.dt.float32
    bf16 = mybir.dt.bfloat16
    i16 = mybir.dt.int16
    EXP = mybir.ActivationFunctionType.Exp
    ADD = mybir.AluOpType.add
    MAX = mybir.AluOpType.max

    tgroups = _half_groups()
    NG = len(tgroups)  # col-slices in the packed stationary
    order = _eng_order()

    nc = bacc.Bacc("TRN2", target_bir_lowering=False, debug=False, num_devices=NCORES)
    # packed E stationary: band r (rows 32r..32r+10) of col-slice g holds the
    # A-rows of the r-th n-tile of group g. Rows outside the bands are unread.
    stat_d = nc.dram_tensor("stat", [P, NG * P], bf16, kind="ExternalInput")
    # E moving: every band holds the same 11 B-rows (coefficient splits).
    mov_d = nc.dram_tensor("mov", [P, MLOC], bf16, kind="ExternalInput")
    rsb_d = nc.dram_tensor("rsb", [P, 4 * NT], bf16, kind="ExternalInput")
    res_d = nc.dram_tensor("res", [3 * MB, MBW], f32, kind="ExternalOutput")

    with tile.TileContext(nc) as tc:
        with (
            tc.tile_pool(name="const", bufs=1) as cpool,
            tc.tile_pool(name="w", bufs=5) as wpool,
            tc.tile_pool(name="epsum", bufs=1, space="PSUM") as epool,
            tc.tile_pool(name="acc", bufs=1, space="PSUM") as apool,
        ):
            # PE warm-up + exp-table preload on a never-written (garbage)
            # tile: no data deps, so both start right after the preamble and
            # run while the input DMAs stream. Results are never read.
            junk = cpool.tile([P, MBW], bf16, tag="junk")
            nc.gpsimd.memset(junk[0:1, 0:1], 0.0)
            ed = epool.tile([P, MBW * 4], f32, tag="e4")
            for _ in range(2):
                nc.tensor.matmul(
                    ed[:, 0:MBW], junk[:, 0:P], junk[:], start=True, stop=True
                )

            # input loads, split across the gpsimd and scalar DMA queues;
            stat = cpool.tile([P, NG * P], bf16)
            mov = cpool.tile([P, MLOC], bf16)
            rsb = cpool.tile([P, 4 * NT], bf16)
            nc.sync.dma_start(mov[:, 0:MBW], mov_d[:, 0:MBW])
            nc.sync.dma_start(stat[:, 0 : 2 * P], stat_d[:, 0 : 2 * P])
            nc.sync.dma_start(rsb[:], rsb_d[:])
            nc.scalar.dma_start(mov[:, MBW:MLOC], mov_d[:, MBW:MLOC])
            # exp-table preload on garbage input; result never read
            scr2 = cpool.tile([1, 8], f32, tag="scr2")
            nc.scalar.activation(scr2[:], junk[0:1, 0:8], EXP)
            # per-partition bias constant for the exp path
            bias_t = cpool.tile([P, 1], f32, tag="bias")
            nc.gpsimd.memset(bias_t[:], BIAS_EXP)
            off = 2
            for cw in [4, 7, 7]:
                nc.gpsimd.dma_start(
                    stat[:, off * P : (off + cw) * P],
                    stat_d[:, off * P : (off + cw) * P],
                )
                off += cw

            # both m-half accumulators share one PSUM bank: rows [s;th;tl]
            # at partitions 0-2 (m-lo) and 32-34 (m-hi, via col tile_position).
            acc = apool.tile([35, MBW], f32)

            started = [False] * MB
            pending = []

            def evict(h):
                st = cpool.tile([3, MBW], f32, tag=f"st{h}")
                nc.vector.tensor_copy(st[:], acc[32 * h : 32 * h + 3, :])
                nc.gpsimd.dma_start(res_d[3 * h : 3 * h + 3, :], st[:])

            def emit_reduce(w, h, tiles):
                for j, i in enumerate(tiles):
                    nc.tensor.matmul(
                        acc[32 * h : 32 * h + 3, :],
                        rsb[:, 4 * i : 4 * i + 3],
                        w[:, j * MBW : (j + 1) * MBW],
                        start=not started[h],
                        stop=i == NT - 1,
                        tile_position=(0, 32 * h),
                    )
                    started[h] = True
                if tiles[-1] == NT - 1:
                    evict(h)

            gi = 0
            for h in range(MB):
                for g, tiles in enumerate(tgroups):
                    if gi % 2 == 0:
                        e = epool.tile([P, MBW * 4], f32, tag="e4")
                    else:
                        e = epool.tile([P, MBW * 3], f32, tag="e3")
                    gi += 1
                    # packed concurrent E matmuls: strip r computes n-tile
                    # tiles[r] using array rows 32r..32r+10.
                    for r, i in enumerate(tiles):
                        nc.tensor.matmul(
                            e[:, r * MBW : (r + 1) * MBW],
                            stat[32 * r : 32 * r + K, g * P : (g + 1) * P],
                            mov[32 * r : 32 * r + K, h * MBW : (h + 1) * MBW],
                            start=True,
                            stop=True,
                            tile_position=(32 * r, 0),
                        )
                    w = wpool.tile([P, MBW * 4], bf16, tag="w")
                    fs = len(tiles) * MBW
                    if order[g] == "S":
                        nc.scalar.activation(
                            w[:, :fs], e[:, :fs], EXP, bias=bias_t[:], scale=SCALE_EXP
                        )
                    else:
                        nc.vector.tensor_scalar(
                            w[:, :fs].bitcast(i16), e[:, :fs], DELTA_SAW, 0.0, ADD, MAX
                        )
                    pending.append((w, h, tiles))
                    if len(pending) > 3:
                        emit_reduce(*pending.pop(0))
            for args in pending:
                emit_reduce(*args)

    nc.compile()
    return nc


def _bf16_split(v):
    import ml_dtypes

    hi = v.astype(ml_dtypes.bfloat16)
    lo = (v - hi.astype(np.float64)).astype(ml_dtypes.bfloat16)
    return hi, lo


@functools.lru_cache(maxsize=1)
def _saw_slots():
    """slot-tile indices (processing order) belonging to saw ('D') groups."""
    order = _eng_order()
    tgroups = _half_groups()
    saw = []
    for g, tiles in enumerate(tgroups):
        if order[g] == "D":
            saw.extend(tiles)
    return saw


def _nperm(outputs):
    """n-permutation: sort by out; saw-group slots get the most-central
    (smallest |out|) tiles, scalar slots the rest."""
    o = outputs.astype(np.float64)
    perm = np.argsort(o)
    o_s = o[perm]
    tile_abs = np.array([np.abs(o_s[t * P : (t + 1) * P]).mean() for t in range(NT)])
    central_rank = np.argsort(tile_abs)  # most-central sorted-tiles first
    saw = _saw_slots()
    exp_slots = [s for s in range(NT) if s not in set(saw)]
    n_saw = len(saw)
    saw_tiles = sorted(central_rank[:n_saw].tolist())
    exp_tiles = sorted(central_rank[n_saw:].tolist())
    tile_of_slot = [None] * NT
    for k_, s in enumerate(sorted(saw)):
        tile_of_slot[s] = saw_tiles[k_]
    for k_, s in enumerate(exp_slots):
        tile_of_slot[s] = exp_tiles[k_]
    return np.concatenate([perm[t * P : (t + 1) * P] for t in tile_of_slot])


def _prepare(x, inputs, outputs, bandwidth):
    """Host-side O(N+M) prep of the factored operands (z16-scaled)."""
    import ml_dtypes

    nperm = _nperm(outputs)
    inputs_p = inputs[nperm]
    o_p = outputs.astype(np.float64)[nperm]

    in0 = inputs_p[:, 0].astype(np.float64)
    in1 = inputs_p[:, 1].astype(np.float64)
    a2 = in0 * in0 + in1 * in1
    x0 = x[:, 0].astype(np.float64)
    x1 = x[:, 1].astype(np.float64)
    b2 = x0 * x0 + x1 * x1
    c = 1.0 / (2.0 * bandwidth.astype(np.float64) ** 2)
    Pm = B16 - K16 * c * b2
    Qm = -K16 * c
    Rm = 2.0 * K16 * c * x0
    Sm = 2.0 * K16 * c * x1

    ones = np.ones(N, np.float64)
    a2h, a2l = _bf16_split(a2)
    i0h, i0l = _bf16_split(in0)
    i1h, i1l = _bf16_split(in1)
    oneh, _ = _bf16_split(ones)
    Ph, Pl = _bf16_split(Pm)
    Qh, Ql = _bf16_split(Qm)
    Rh, Rl = _bf16_split(Rm)
    Sh, Sl = _bf16_split(Sm)

    # row pairing: z = P(hi+lo) + a2hi*Q(hi+lo) + a2lo*Qhi + (same for in0,in1)
    stat_rows = np.stack(
        [oneh, oneh, a2h, a2h, a2l, i0h, i0h, i0l, i1h, i1h, i1l]
    )  # (K, N)
    mov_rows = np.stack([Ph, Pl, Qh, Ql, Qh, Rh, Rl, Rh, Sh, Sl, Sh])  # (K, M)

    tgroups = _half_groups()
    NG = len(tgroups)
    stat = np.zeros((P, NG * P), ml_dtypes.bfloat16)
    for g, tiles in enumerate(tgroups):
        for r, i in enumerate(tiles):
            stat[32 * r : 32 * r + K, g * P : (g + 1) * P] = stat_rows[
                :, i * P : (i + 1) * P
            ]
    mov = np.zeros((P, M), ml_dtypes.bfloat16)
    for r in range(4):
        mov[32 * r : 32 * r + K, :] = mov_rows

    oh, ol = _bf16_split(o_p)
    rsb = np.zeros((N, 4), ml_dtypes.bfloat16)
    rsb[:, 0] = 1.0
    rsb[:, 1] = oh
    rsb[:, 2] = ol
    rsb_sb = np.ascontiguousarray(
        rsb.reshape(NT, P, 4).transpose(1, 0, 2).reshape(P, 4 * NT)
    )
    return stat, mov, rsb_sb


def _in_maps(stat, mov, rsb_sb):
    return [
        {
            "stat": np.ascontiguousarray(stat),
            "mov": np.ascontiguousarray(mov[:, c * MLOC : (c + 1) * MLOC]),
            "rsb": np.ascontiguousarray(rsb_sb),
        }
        for c in range(NCORES)
    ]


def kernel(x, inputs, outputs, bandwidth):
    from concourse.bass_utils import run_bass_kernel_spmd

    x = np.asarray(x, np.float32)
    inputs = np.asarray(inputs, np.float32)
    outputs = np.asarray(outputs, np.float32)
    bandwidth = np.asarray(bandwidth, np.float32)

    stat, mov, rsb_sb = _prepare(x, inputs, outputs, bandwidth)

    nc = _build()
    in_maps = _in_maps(stat, mov, rsb_sb)
    try:
        res = run_bass_kernel_spmd(nc, in_maps, list(range(NCORES)))
    except Exception:
        # transient NRT_EXEC_UNIT_UNRECOVERABLE after an interrupted prior
        # run; the device recovers after a short wait.
        import time

        time.sleep(20)
        res = run_bass_kernel_spmd(nc, in_maps, list(range(NCORES)))
    parts = []
    for c in range(NCORES):
        st = res.results[c]["res"]  # (6,512): [s,th,tl] x {m-lo, m-hi}
        s = np.concatenate([st[0], st[3]]).astype(np.float64)
        t = np.concatenate(
            [st[1].astype(np.float64) + st[2], st[4].astype(np.float64) + st[5]]
        )
        parts.append(t / s)
    return np.concatenate(parts).astype(np.float32)


if __name__ == "__main__":
    rng = np.random.default_rng(0)
    x = rng.standard_normal((M, 2), np.float32)
    inputs = rng.standard_normal((N, 2), np.float32)
    outputs = rng.standard_normal(N, np.float32)
    bandwidth = (0.5 + rng.random(M)).astype(np.float32)
    got = kernel(x, inputs, outputs, bandwidth)
    print(got[:8])
